# revision 16
# baseline (speedup 1.0000x reference)
"""Radius neighbor search (ball query) on 8 Trainium2 NeuronCores.

Strategy (hardcoded for data=[12000,3], queries=[12000,3], radius scalar):
  - Shard queries across the 8 cores (1500 rows each); replicate data.
  - dist2 = ||q||^2 + ||d||^2 - 2 q.d is computed as ONE K=30 bf16 matmul per
    tile: every f32 factor of the 5-term augmented form
    ([q2, 1, -2q] . [1, d2, d]) is split into 3 bf16 terms; all kept partial
    products are exact in the PE's f32 accumulator (only ~2^-32-relative
    terms are dropped), so precision matches a plain f32 matmul while the PE
    streams at bf16 rate (fp32 matmul is 4x slower and weight-load-bound).
  - Per 128-query chunk (split into `NSPLIT` column blocks for pipelining):
    PE writes dist2 into PSUM (512-col matmuls), ACT does relu(dist2)
    PSUM->SBUF (the clamp), DVE computes mask = (w <= r^2) as u8 with fused
    per-row counts (accum_out), then weights = (w <= r^2) * w in-place
    (scalar_tensor_tensor).
  - Outputs per core: weights [1500,12000] f32, mask [1500,12000] u8,
    counts [128, NCHUNKS*NSPLIT] f32.  Host does the tiny cross-shard
    exclusive scan for row_splits and assembles full outputs.
"""

import math

import numpy as np

import concourse.bacc as bacc
import concourse.mybir as mybir
import concourse.tile as tile
from concourse.bass_utils import run_bass_kernel_spmd

# Problem constants (fixed by the harness).
Q = 12000
N = 12000
DIM = 3
CORES = 8
QS = Q // CORES  # queries per core
P = 128  # partitions
K = 30  # augmented contraction dim (3-way bf16 split, see make_in_maps)
PSUM_W = 2048  # psum tile width (4 banks)
MM_W = 512  # max matmul moving free dim (1 psum bank)
NCHUNKS = math.ceil(QS / P)  # 12 query chunks per core

# Pipelining knobs
NSPLIT = 6  # column blocks per chunk
WORK_BUFS = 8  # buffers for w/mask tiles
MASK_ENGINE = "vector"  # "vector" | "gpsimd"

ALL_STAGES = frozenset({"mm", "act", "mask", "stt", "store"})

F32 = mybir.dt.float32
BF16 = mybir.dt.bfloat16
U8 = mybir.dt.uint8


def build_program(repeat: int = 1, nsplit: int = None, work_bufs: int = None,
                  mask_engine: str = None, psum_w: int = None,
                  stages: frozenset = ALL_STAGES):
    """Build + compile the per-core Bass program. Returns the Bacc module.

    `stages` subsets {"mm","act","mask","stt","store"} (or {"dma_only"}) for
    timing experiments.
    """
    nsplit = NSPLIT if nsplit is None else nsplit
    work_bufs = WORK_BUFS if work_bufs is None else work_bufs
    mask_engine = MASK_ENGINE if mask_engine is None else mask_engine
    psum_w = PSUM_W if psum_w is None else psum_w
    assert N % nsplit == 0
    nb = N // nsplit  # columns per block
    dma_only = "dma_only" in stages

    nc = bacc.Bacc("TRN2", target_bir_lowering=False, debug=False)

    qaug_d = nc.dram_tensor("qaug", [K, QS], BF16, kind="ExternalInput")
    daug_d = nc.dram_tensor("daug", [K, N], BF16, kind="ExternalInput")
    r2_d = nc.dram_tensor("r2", [P, 1], F32, kind="ExternalInput")
    w_d = nc.dram_tensor("weights", [QS, N], F32, kind="ExternalOutput")
    m_d = nc.dram_tensor("mask", [QS, N], U8, kind="ExternalOutput")
    c_d = nc.dram_tensor("counts", [P, NCHUNKS * nsplit], F32, kind="ExternalOutput")

    def mask_eng_for(idx):
        if mask_engine == "alternate":
            return nc.vector if idx % 2 == 0 else nc.gpsimd
        return nc.vector if mask_engine == "vector" else nc.gpsimd

    with tile.TileContext(nc) as tc:
        with (
            tc.tile_pool(name="const", bufs=1) as const_pool,
            tc.tile_pool(name="work", bufs=work_bufs) as work,
            tc.tile_pool(name="psum", bufs=(16 * 1024 // (psum_w * 4)),
                         space="PSUM") as psum,
        ):
            qaug_sb = const_pool.tile([K, QS], BF16)
            nc.sync.dma_start(qaug_sb[:, :], qaug_d[:, :])
            daug_sb = const_pool.tile([K, N], BF16)
            nc.sync.dma_start(daug_sb[:, :], daug_d[:, :])
            r2_sb = const_pool.tile([P, 1], F32)
            nc.sync.dma_start(r2_sb[:, :], r2_d[:, :])
            counts_sb = const_pool.tile([P, NCHUNKS * nsplit], F32)
            nc.vector.memset(counts_sb[:, :], 0.0)

            for c in [c for _ in range(repeat) for c in range(NCHUNKS)]:
                r0 = c * P
                rows = min(P, QS - r0)
                for h in range(nsplit):
                    n0 = h * nb
                    w_sb = work.tile([P, nb], F32, tag="w")
                    mask_sb = work.tile([P, nb], U8, tag="mask")

                    if dma_only:
                        nc.vector.memset(w_sb[:rows, :1], 0.0)
                        nc.vector.memset(mask_sb[:rows, :1], 0)
                        nc.sync.dma_start(
                            w_d[r0 : r0 + rows, n0 : n0 + nb], w_sb[:rows, :]
                        )
                        nc.sync.dma_start(
                            m_d[r0 : r0 + rows, n0 : n0 + nb], mask_sb[:rows, :]
                        )
                        continue

                    # dist2 into PSUM, relu'd out to SBUF per psum slab
                    for off in range(0, nb, psum_w):
                        width = min(psum_w, nb - off)
                        pt = psum.tile([P, psum_w], F32, tag="pt")
                        if "mm" in stages:
                            for off2 in range(0, width, MM_W):
                                mw = min(MM_W, width - off2)
                                nc.tensor.matmul(
                                    pt[:rows, off2 : off2 + mw],
                                    qaug_sb[:, r0 : r0 + rows],
                                    daug_sb[
                                        :, n0 + off + off2 : n0 + off + off2 + mw
                                    ],
                                )
                        if "act" in stages:
                            nc.scalar.activation(
                                w_sb[:rows, off : off + width],
                                pt[:rows, :width],
                                mybir.ActivationFunctionType.Relu,
                            )

                    # mask (u8) + per-row counts in one pass
                    if "mask" in stages:
                        mask_eng_for(c * nsplit + h).tensor_scalar(
                            out=mask_sb[:rows, :],
                            in0=w_sb[:rows, :],
                            scalar1=r2_sb[:rows, :],
                            scalar2=None,
                            op0=mybir.AluOpType.is_le,
                            op1=mybir.AluOpType.add,  # accum_out = add-reduce(out)
                            accum_out=counts_sb[
                                :rows, c * nsplit + h : c * nsplit + h + 1
                            ],
                        )
                    # weights = (w <= r^2) * w, in place
                    if "stt" in stages:
                        nc.vector.scalar_tensor_tensor(
                            out=w_sb[:rows, :],
                            in0=w_sb[:rows, :],
                            scalar=r2_sb[:rows, :],
                            in1=w_sb[:rows, :],
                            op0=mybir.AluOpType.is_le,
                            op1=mybir.AluOpType.mult,
                        )

                    if "store" in stages:
                        nc.sync.dma_start(
                            w_d[r0 : r0 + rows, n0 : n0 + nb], w_sb[:rows, :]
                        )
                        nc.sync.dma_start(
                            m_d[r0 : r0 + rows, n0 : n0 + nb], mask_sb[:rows, :]
                        )

            if "mask" in stages or dma_only:
                nc.sync.dma_start(c_d[:, :], counts_sb[:, :])

    nc.compile()
    return nc


_CACHED_NC = None


def get_program():
    global _CACHED_NC
    if _CACHED_NC is None:
        _CACHED_NC = build_program()
    return _CACHED_NC


def make_in_maps(data, queries, radius):
    """Host-side prep: augmented 5-dim vectors + per-core query shards."""
    data = np.ascontiguousarray(np.asarray(data, dtype=np.float32))
    queries = np.ascontiguousarray(np.asarray(queries, dtype=np.float32))
    r = np.float32(np.asarray(radius).reshape(()))

    import ml_dtypes

    bf16 = ml_dtypes.bfloat16

    def split3(v):
        """Split f32 vector into 3 bf16 terms summing (near-)exactly to v."""
        v = v.astype(np.float32)
        v0 = v.astype(bf16)
        r1 = v - v0.astype(np.float32)
        v1 = r1.astype(bf16)
        r2 = r1 - v1.astype(np.float32)
        v2 = r2.astype(bf16)
        return v0, v1, v2

    q2 = np.sum(queries * queries, axis=1, dtype=np.float32)  # [Q]
    d2 = np.sum(data * data, axis=1, dtype=np.float32)  # [N]
    ones_q = np.ones(Q, bf16)
    ones_d = np.ones(N, bf16)

    # dist2 = sum_k qaug[k, q] * daug[k, n], with every f32 factor split into
    # 3 bf16 terms; all kept products are exact in the PE's f32 accumulator.
    # Per f32xf32 product (a0+a1+a2)(b0+b1+b2) we keep 8 partial products and
    # drop only a2*b2 (<= 2^-32 relative).
    qrows, drows = [], []
    # q2 * 1
    for t in split3(q2):
        qrows.append(t)
        drows.append(ones_d)
    # 1 * d2
    for t in split3(d2):
        qrows.append(ones_q)
        drows.append(t)
    # -2 q_c * d_c
    for c in range(3):
        a0, a1, a2 = split3(-2.0 * queries[:, c])
        b0, b1, b2 = split3(data[:, c])
        for qa, db in [(a0, b0), (a0, b1), (a1, b0), (a0, b2),
                       (a2, b0), (a1, b1), (a1, b2), (a2, b1)]:
            qrows.append(qa)
            drows.append(db)
    qaug = np.stack(qrows, axis=0).astype(bf16)  # [30, Q]
    daug = np.stack(drows, axis=0).astype(bf16)  # [30, N]

    r2_tile = np.full((P, 1), r * r, np.float32)

    in_maps = []
    for c in range(CORES):
        sl = slice(c * QS, (c + 1) * QS)
        in_maps.append(
            {
                "qaug": np.ascontiguousarray(qaug[:, sl]),
                "daug": daug,
                "r2": r2_tile,
            }
        )
    return in_maps


def assemble(results, nsplit: int = None):
    """Gather per-core results into full outputs (mask, row_splits, weights)."""
    nsplit = NSPLIT if nsplit is None else nsplit
    mask = np.empty((Q, N), np.bool_)
    weights = np.empty((Q, N), np.float32)
    counts = np.empty(Q, np.int64)
    for c, res in enumerate(results):
        sl = slice(c * QS, (c + 1) * QS)
        mask[sl] = res["mask"].astype(np.bool_)
        weights[sl] = res["weights"]
        # counts tile is [P, NCHUNKS*nsplit]; query q = chunk*P + p owns the
        # nsplit columns [chunk*nsplit, (chunk+1)*nsplit)
        ct = res["counts"].reshape(P, NCHUNKS, nsplit).sum(axis=2)  # [P, NCHUNKS]
        cc = ct.T.reshape(-1)[:QS]
        counts[sl] = cc.astype(np.int64)
    row_splits = np.concatenate(
        [np.zeros(1, np.int64), np.cumsum(counts)]
    ).astype(np.int32)
    return mask, row_splits, weights


def kernel(data, queries, radius):
    nc = get_program()
    in_maps = make_in_maps(data, queries, radius)
    last_err = None
    for _ in range(3):  # retry transient device errors
        try:
            res = run_bass_kernel_spmd(nc, in_maps, core_ids=list(range(CORES)))
            return assemble(res.results)
        except Exception as e:  # noqa: BLE001
            last_err = e
    raise last_err
